# revision 45
# baseline (speedup 1.0000x reference)
"""Trainium2 Bass kernel for nn_AttentionSpikingNetwork (B=64, S=512).

Data-parallel over batch across 8 NeuronCores (8 batch elems per core).
All matmuls run as float32r (FP22, full PE rate). PE-work-minimized pass
structure (validated against the reference inputs in a numpy FP22 emulator;
rel err 2.6e-3, zero spike flips, stable under accumulation-order noise):
  - embed (threshold-critical): exact 3-term hi/lo (Weh*xh + Weh*xl +
    Wel*xh), emitted as one packed K=2352 stream [xh; xl; xh] against
    [Weh; Weh; Wel] so partial chunks merge: 95 matmuls/elem instead of
    the naive 105.
  - Q/K: one packed single-pass matmul group (Q cols 0:64, K cols 64:128);
    the K rows are DMA-shifted to partitions 0:64 of persistent zero-padded
    tiles so scores stay in the fast K=128 tile mode.
  - scores: single pass (softmax normalization cancels FP22 rounding).
  - V: single pass (Wv-hi only); the dropped s1*Wv-lo term's batch mean is
    folded into bv on the host (spike rates from a cheap host embed
    forward), measured 2.4x error reduction.
  - attention: single pass on the FP22-rounded V (the rounding noise is
    averaged away by the attention weights; measured exact-class).
  - cur2 (threshold-critical): exact 3-pass hi/lo.
  - cur3: exact 2-pass (spikes are FP22-exact).
Activations flow transposed ([feat, seq]) so biases/thresholds fuse into
single per-partition DVE ops reading PSUM. Scores are produced transposed
(K @ Q.T); the softmax runs without max-subtraction and its denominator
comes from PE ones-matmuls off the critical path. Batch element b+1's
embed chunk-groups are interleaved through b's entire attention phase so
x-chunk consumption matches the uniform DMA delivery rate and every
latency chain (exp, reciprocal, normalize, spike drains) hides under
embed matmuls; the last element's qk/V/scores/den are hoisted into the
previous phase so only its attention tail remains unoverlapped. DMA
trigger queues are chosen so nothing latency-critical (exp on scalar,
invb broadcast on gpsimd) sits behind bulk transfer triggers.
"""
import os
import sys

for _p in ("/opt/trn_rl_repo", "/root/.axon_site/_ro/trn_rl_repo"):
    if os.path.isdir(_p) and _p not in sys.path:
        sys.path.insert(0, _p)

import numpy as np
from contextlib import ExitStack

import concourse.bass as bass
import concourse.bass_isa as bass_isa
import concourse.bacc as bacc
import concourse.mybir as mybir
import concourse.tile as tile
from concourse.bass_utils import run_bass_kernel_spmd

F32 = mybir.dt.float32
F32R = mybir.dt.float32r
AF = mybir.ActivationFunctionType
OP = mybir.AluOpType

NCORES = 8
B, S, DIN, DEMB, DQK, DH2, DOUT = 64, 512, 784, 600, 64, 200, 10
NB = B // NCORES  # batch elems per core

def _chunks(total, step=128):
    return [(i, min(step, total - i)) for i in range(0, total, step)]

KSTACK = 3 * DIN          # packed [xh; xl; xh] contraction length
CH_KS = _chunks(KSTACK)   # 19 chunks (18x128 + 48)
CH_EMB = _chunks(DEMB)    # 5
CH_H2 = _chunks(DH2)      # 2
CH_S = _chunks(S)         # 4
CH_VN = [(0, 344), (344, 256)]  # V free-dim split; both >=256 keeps fp32r full-rate


def round_m11(a):
    """Round fp32 to 11 explicit mantissa bits (fp32r/FP22 grid), RNE."""
    a = np.ascontiguousarray(a, np.float32)
    u = a.view(np.uint32).astype(np.uint64)
    r = (u + 0x7FF + ((u >> 12) & 1)) & np.uint64(0xFFFFF000)
    return r.astype(np.uint32).view(np.float32)


def _split(a):
    hi = round_m11(a)
    lo = (a.astype(np.float32) - hi).astype(np.float32)
    return hi, lo


def build_nc(nb=NB):
    nc = bacc.Bacc()

    def par(name, shape, dt=F32R, out=False):
        return nc.declare_dram_parameter(name, list(shape), dt, isOutput=out)

    xpk = par("xpk", [nb, KSTACK, S])
    wS = par("wS", [KSTACK * DEMB])
    wQK = par("wQK", [DEMB, 128])
    wVh = par("wVh", [DEMB, DEMB])
    w2h = par("w2h", [DEMB * DH2]); w2l = par("w2l", [DEMB * DH2])
    w3h = par("w3h", [DH2, DOUT]); w3l = par("w3l", [DH2, DOUT])
    bE = par("bE", [DEMB, 1], F32); bQK = par("bQK", [128, 1], F32)
    bV = par("bV", [DEMB, 1], F32)
    b2 = par("b2", [DH2, 1], F32); b3 = par("b3", [DOUT, 1], F32)
    ones = par("ones", [128, 1])
    z64 = par("z64", [64, 128])
    os_ = par("os", [nb, DOUT, S], F32, out=True)
    om_ = par("om", [nb, DOUT, S], F32, out=True)

    with ExitStack() as ctx:
        tc = ctx.enter_context(tile.TileContext(nc))
        wp = ctx.enter_context(tc.tile_pool(name="wp", bufs=1))
        xp = ctx.enter_context(tc.tile_pool(name="xp", bufs=12))
        sp = ctx.enter_context(tc.tile_pool(name="sp", bufs=1))
        outp = ctx.enter_context(tc.tile_pool(name="outp", bufs=1))
        ps_em = ctx.enter_context(tc.tile_pool(name="ps_em", bufs=1, space="PSUM"))
        ps = ctx.enter_context(tc.tile_pool(name="ps", bufs=3, space="PSUM"))

        # ---- resident weights / consts ----
        # DMA emission order is load order: the packed embed weight blocks
        # stream in per-k-chunk interleaved with b=0's x chunks so the first
        # matmul starts after ~0.5MB. Everything else loads during b=0's
        # embed compute (see _load_rest below).
        def blocks2(dram, rchs, cchs, nm, dma=True):
            """dedicated [rn, cn] weight blocks, host-packed contiguously"""
            out = {}
            off = 0
            for i, (r0, rn) in enumerate(rchs):
                for j, (c0, cn) in enumerate(cchs):
                    t = wp.tile([rn, cn], F32R, name=f"{nm}_{i}_{j}",
                                tag=f"{nm}_{i}_{j}")
                    out[(i, j)] = (t, off, rn, cn)
                    if dma:
                        nc.scalar.dma_start(
                            out=t, in_=dram[off:off + rn * cn].rearrange(
                                "(a b) -> a b", b=cn))
                    off += rn * cn
            return out

        def wtiles(dram, chs, width, nm, dma=True):
            hs = []
            for i, (c0, cn) in enumerate(chs):
                t = wp.tile([cn, width], F32R, name=f"{nm}{i}", tag=f"{nm}{i}")
                if dma:
                    nc.scalar.dma_start(out=t, in_=dram[c0:c0 + cn, :])
                hs.append(t)
            return hs

        def btiles(dram, chs, nm):
            hs = []
            for i, (c0, cn) in enumerate(chs):
                t = wp.tile([cn, 1], F32, name=f"{nm}{i}", tag=f"{nm}{i}")
                nc.scalar.dma_start(out=t, in_=dram[c0:c0 + cn, :])
                hs.append(t)
            return hs

        wS_m = blocks2(wS, CH_KS, CH_EMB, "wS", dma=False)
        _rest = {}

        def _load_rest():
            _rest["wQK"] = wtiles(wQK, CH_EMB, 128, "wQK")
            _rest["bQK"] = btiles(bQK, [(0, 128)], "bQK")[0]
            _rest["wVh"] = wtiles(wVh, CH_EMB, DEMB, "wVh")
            _rest["bV"] = btiles(bV, CH_EMB, "bV")
            _rest["w2h"] = {k: v[0] for k, v in
                            blocks2(w2h, CH_EMB, CH_H2, "w2h").items()}
            _rest["w2l"] = {k: v[0] for k, v in
                            blocks2(w2l, CH_EMB, CH_H2, "w2l").items()}
            _rest["b2"] = btiles(b2, CH_H2, "b2")
            _rest["w3h"] = wtiles(w3h, CH_H2, DOUT, "w3h")
            _rest["w3l"] = wtiles(w3l, CH_H2, DOUT, "w3l")
            _rest["b3"] = btiles(b3, [(0, DOUT)], "b3")[0]

        bE_t = btiles(bE, CH_EMB, "bE")
        ones_t = wp.tile([128, 1], F32R, name="ones_t", tag="ones_t")
        nc.scalar.dma_start(out=ones_t, in_=ones[:, :])

        # Persistent scores-lhsT tiles: rows 0:64 get each elem's K slice
        # (DMA'd down from the packed QK drain), rows 64:128 stay zero so
        # the scores matmul runs at the fast K=128 tile mode.
        kh_t = []
        for j, (t0, tn) in enumerate(CH_S):
            kh = wp.tile([128, tn], F32R, name=f"kh{j}", tag=f"kh{j}")
            nc.scalar.dma_start(out=kh[64:128, :], in_=z64[:, 0:tn])
            kh_t.append(kh)

        MM = nc.tensor.matmul

        # Software pipeline: elem b+1's embed matmuls are emitted between
        # elem b's scores and its softmax-sum/attention matmuls, giving the
        # PE ~20us of independent work while ACT/DVE run b's exp chain.
        st = [dict() for _ in range(nb)]

        def emit_embed_start(b):
            em_ps = []
            for i, (c0, cn) in enumerate(CH_EMB):
                t = ps_em.tile([cn, S], F32, name=f"em{i}", tag=f"em{i}")
                em_ps.append(t)
            st[b]["em_ps"] = em_ps
            st[b]["xt"] = {}

        def prefetch_x(b, q, kidx):
            # issue x-chunk DMAs well ahead of their matmuls. Allocation in
            # consumption order keeps the xp buffer cycle aligned with the
            # matmul order (chunk k+11 reuses chunk k's buffer, which is
            # long consumed). The gpsimd-queue portion is issued only after
            # den: a DMA trigger occupies its queue for ~0.7us, and the
            # latency-critical invb broadcast must not sit behind them.
            xt = st[b]["xt"]
            for k in kidx:
                k0, kn = CH_KS[k]
                t = xp.tile([kn, S], F32R, name=f"x{k}", tag="x_t")
                q.dma_start(out=t, in_=xpk[b, k0:k0 + kn, :])
                xt[k] = t

        def emit_embed_stack(b, kidx):
            em_ps = st[b]["em_ps"]
            last = len(CH_KS) - 1
            for k in kidx:
                if b == 0:
                    # b=0 is HBM-bound (~10.4MB before the embed ends), so
                    # emission order is per-chunk across all three queues:
                    # weight blocks alternate scalar/gpsimd, x chunks
                    # alternate sync/gpsimd, so every queue's delivery of
                    # chunk k's data slightly precedes the PE's need for it
                    for j in range(len(CH_EMB)):
                        t, off, rn, cn_ = wS_m[(k, j)]
                        q = nc.scalar if (k + j) % 2 == 0 else nc.gpsimd
                        q.dma_start(
                            out=t, in_=wS[off:off + rn * cn_].rearrange(
                                "(a b) -> a b", b=cn_))
                    k0, kn = CH_KS[k]
                    t = xp.tile([kn, S], F32R, name=f"x{k}", tag="x_t")
                    (nc.sync if k % 2 == 0 else nc.gpsimd).dma_start(
                        out=t, in_=xpk[b, k0:k0 + kn, :])
                    st[b]["xt"][k] = t
                x_t = st[b]["xt"][k]
                for j in range(len(CH_EMB)):
                    MM(em_ps[j], wS_m[(k, j)][0], x_t,
                       start=(k == 0), stop=(k == last))
            if b == 0 and 0 in kidx:
                _load_rest()

        def emit_embed_drain(b):
            em_ps = st[b]["em_ps"]
            s1_t = []
            for i, (c0, cn) in enumerate(CH_EMB):
                t = sp.tile([cn, S], F32R, name=f"s1_{i}", tag=f"s1_{i}", bufs=2)
                nc.vector.tensor_scalar(t, em_ps[i], bE_t[i], 0.5, OP.add, OP.is_gt)
                s1_t.append(t)
            st[b]["s1"] = s1_t

        def emit_qk(b):
            s1_t = st[b]["s1"]
            wQK_t = _rest["wQK"]

            # Packed Q|K single pass (Q cols 0:64, K cols 64:128): one
            # 5-matmul group instead of two. Scores single-pass FP22 (the
            # softmax normalization cancels the common-mode rounding).
            qk_ps = ps.tile([128, S], F32, name="qk_ps", tag="ps")
            n = len(CH_EMB)
            for i in range(n):
                MM(qk_ps, wQK_t[i], s1_t[i], start=(i == 0),
                   stop=(i == n - 1))
            qh_t = sp.tile([128, S], F32R, name="qh", tag="qh")
            nc.vector.tensor_scalar(qh_t, qk_ps, _rest["bQK"], None, OP.add)
            # K rows shift down to partitions 0:64 of the persistent kh
            # tiles (rows 64:128 zero); qh_t itself is the scores rhs — its
            # K rows 64:128 meet the kh zeros. Triggered from the sync
            # queue, ahead of the x prefetch: the trigger blocks its queue
            # until the drain lands, and both the scalar queue (exp) and
            # gpsimd queue (invb broadcast) have latency-critical work.
            for j, (t0, tn) in enumerate(CH_S):
                nc.sync.dma_start(out=kh_t[j][0:64, :],
                                  in_=qh_t[64:128, t0:t0 + tn])

            st[b].update(kh=kh_t, qh=qh_t)

        def emit_V(b, chs=None, append=False):
            s1_t = st[b]["s1"]
            wVh_t = _rest["wVh"]
            # V natural = spk1 @ Wvh.T (single pass; the dropped Wv-lo term's
            # mean is compensated in bV host-side). QK psum drains hide here.
            vh_t = st[b]["vh"] if append else []
            base = len(vh_t)
            for dti, (t0, tn) in enumerate(chs if chs is not None else CH_S):
                ti = base + dti
                v_ps = [ps.tile([tn, w], F32, name=f"v_ps{j}", tag="ps")
                        for j, (v0, w) in enumerate(CH_VN)]
                n = len(CH_EMB)
                for i in range(n):
                    lh = s1_t[i][:, t0:t0 + tn]
                    for j, (v0, w) in enumerate(CH_VN):
                        MM(v_ps[j], lh, wVh_t[i][:, v0:v0 + w],
                           start=(i == 0), stop=(i == n - 1))
                vh = sp.tile([tn, DEMB], F32R, name=f"vh{ti}", tag=f"vh{ti}")
                for j, (v0, w) in enumerate(CH_VN):
                    nc.vector.tensor_copy(vh[:, v0:v0 + w], v_ps[j])
                vh_t.append(vh)

            st[b]["vh"] = vh_t

        def emit_scores(b):
            qh_t, kh_t = st[b]["qh"], st[b]["kh"]
            # scores.T = K @ Q.T (single-pass FP22) + exp, per t-chunk.
            # exp writes the f32r attention operand directly from PSUM.
            pth_t = []
            for ti, (t0, tn) in enumerate(CH_S):
                scT_ps = ps.tile([tn, S], F32, name=f"scT_ps{ti}", tag="ps")
                MM(scT_ps, kh_t[ti], qh_t, start=True, stop=True)
                ph = sp.tile([tn, S], F32R, name=f"pth{ti}", tag=f"pth{ti}")
                nc.scalar.activation(ph, scT_ps, AF.Exp, scale=0.125)
                pth_t.append(ph)
            st[b].update(pth=pth_t)

        def emit_den(b):
            # softmax denominator = Sum_t P.T via PE ones-matmuls (the same
            # rounded operand the attention numerator uses). Emitted
            # mid-embed-filler so the slow DVE reciprocal lands well before
            # the spk2_in stage needs invb.
            pth_t = st[b]["pth"]
            den_ps = ps.tile([1, S], F32, name="den_ps", tag="ps")
            nt = len(CH_S)
            for ti in range(nt):
                MM(den_ps, ones_t[0:CH_S[ti][1], :], pth_t[ti],
                   start=(ti == 0), stop=(ti == nt - 1))
            invs = sp.tile([1, S], F32, name="invs", tag="invs", bufs=2)
            nc.vector.reciprocal(invs, den_ps)
            invb = sp.tile([128, S], F32, name="invb", tag="invb")
            nc.gpsimd.partition_broadcast(invb, invs)
            st[b]["invb"] = invb

        # attn/cur2 chunk order: the 88-row chunk first, so its DVE chain
        # (the last cur2 passes depend on it) starts earliest
        IORDER = [len(CH_EMB) - 1] + list(range(len(CH_EMB) - 1))

        def emit_attn_chunk(b, i):
            # attn_out.T = V.T @ P.T (single pass); + normalize + bv + spk1.T
            vh_t, pth_t = st[b]["vh"], st[b]["pth"]
            invb = st[b]["invb"]
            nt = len(CH_S)
            c0, cn = CH_EMB[i]
            ao_ps = ps.tile([cn, S], F32, name=f"ao_ps{i}", tag="ps")
            for ti in range(nt):
                MM(ao_ps, vh_t[ti][:, c0:c0 + cn], pth_t[ti],
                   start=(ti == 0), stop=(ti == nt - 1))
            raw = sp.tile([cn, S], F32, name="s2raw", tag="s2raw", bufs=2)
            nc.vector.scalar_tensor_tensor(raw, ao_ps, 0.0, invb[0:cn, :],
                                           OP.add, OP.mult)
            nc.vector.scalar_tensor_tensor(raw, raw, _rest["bV"][i],
                                           st[b]["s1"][i].bitcast(F32),
                                           OP.add, OP.add)
            h = sp.tile([cn, S], F32R, name=f"s2h{i}", tag=f"s2h{i}")
            l = sp.tile([cn, S], F32R, name=f"s2l{i}", tag=f"s2l{i}")
            nc.vector.tensor_copy(h, raw)
            nc.vector.tensor_tensor(l, raw, h.bitcast(F32), OP.subtract)
            st[b].setdefault("s2h", {})[i] = h
            st[b].setdefault("s2l", {})[i] = l

        def emit_cur2_part(b, hi):
            # cur2.T = W2 @ spk2_in.T (exact 3 passes) -> spk2
            s2h_t, s2l_t = st[b]["s2h"], st[b]["s2l"]
            h0, hn = CH_H2[hi]
            c2_ps = ps.tile([hn, S], F32, name=f"c2_ps{hi}", tag="ps")
            for oi, i in enumerate(IORDER):
                wh = _rest["w2h"][(i, hi)]
                wl = _rest["w2l"][(i, hi)]
                MM(c2_ps, wh, s2h_t[i], start=(oi == 0), stop=False)
                MM(c2_ps, wh, s2l_t[i], start=False, stop=False)
                MM(c2_ps, wl, s2h_t[i],
                   start=False, stop=(oi == len(IORDER) - 1))
            t = sp.tile([hn, S], F32R, name=f"spk2_{hi}", tag=f"spk2_{hi}")
            nc.vector.tensor_scalar(t, c2_ps, _rest["b2"][hi], 0.3,
                                    OP.add, OP.is_gt)
            st[b].setdefault("s2", {})[hi] = t

        def emit_cur3_out(b):
            # cur3.T = W3 @ spk2.T (exact 2 passes) -> outputs
            s2_t = st[b]["s2"]
            c3_ps = ps.tile([DOUT, S], F32, name="c3_ps", tag="ps")
            n = len(CH_H2)
            for hi in range(n):
                MM(c3_ps, _rest["w3h"][hi], s2_t[hi], start=(hi == 0), stop=False)
                MM(c3_ps, _rest["w3l"][hi], s2_t[hi], start=False, stop=(hi == n - 1))
            spk3_t = outp.tile([DOUT, S], F32, name="spk3_t", tag="spk3_t")
            c3b_t = outp.tile([DOUT, S], F32, name="c3b_t", tag="c3b_t")
            mem3_t = outp.tile([DOUT, S], F32, name="mem3_t", tag="mem3_t")
            nc.vector.tensor_scalar(spk3_t, c3_ps, _rest["b3"], 0.3, OP.add, OP.is_gt)
            nc.vector.tensor_scalar(c3b_t, c3_ps, _rest["b3"], None, OP.add)
            nc.vector.scalar_tensor_tensor(mem3_t, spk3_t, -0.3, c3b_t,
                                           OP.mult, OP.add)
            nc.scalar.dma_start(out=os_[b, :, :], in_=spk3_t)
            nc.scalar.dma_start(out=om_[b, :, :], in_=mem3_t)

        NKS = len(CH_KS)
        emit_embed_start(0)
        emit_embed_stack(0, range(NKS))
        emit_embed_drain(0)
        for b in range(nb):
            if b == nb - 1:
                # qk/V/scores/den were hoisted into b-1's phase; only the
                # attention tail remains here.
                for i in IORDER:
                    emit_attn_chunk(b, i)
                emit_cur2_part(b, 0)
                emit_cur2_part(b, 1)
                emit_cur3_out(b)
                break
            emit_qk(b)
            # issue b+1's first 11 x DMAs now: the PE consumes them
            # ~35us from here, so the sync queue never starves the
            # embed matmuls. The rest go on gpsimd after den.
            emit_embed_start(b + 1)
            prefetch_x(b + 1, nc.sync, range(11))
            emit_V(b)
            # b+1's embed chunk-groups are interleaved through b's whole
            # attention phase: x-chunk consumption is then spread evenly,
            # matching the uniform DMA delivery rate, and every latency
            # chain (exp, reciprocal, normalize, spike drains) hides under
            # embed matmuls.
            emit_embed_stack(b + 1, [0])
            emit_scores(b)
            emit_embed_stack(b + 1, [1])
            emit_den(b)
            prefetch_x(b + 1, nc.gpsimd, range(11, NKS))
            emit_embed_stack(b + 1, [2, 3])
            for n_at, i in enumerate(IORDER):
                emit_attn_chunk(b, i)
                emit_embed_stack(b + 1, range(4 + 2 * n_at, 6 + 2 * n_at))
            emit_cur2_part(b, 0)
            emit_embed_stack(b + 1, range(14, 17))
            emit_cur2_part(b, 1)
            emit_embed_stack(b + 1, range(17, NKS))
            emit_cur3_out(b)
            emit_embed_drain(b + 1)
            if b + 1 == nb - 1:
                # hoist the last element's pre-attention stages here so its
                # exp/reciprocal chains hide under this phase's matmuls
                emit_qk(b + 1)
                emit_V(b + 1)
                emit_scores(b + 1)
                emit_den(b + 1)

    nc.finalize()
    return nc


_NC_CACHE = {}


def _get_nc(nb):
    if nb not in _NC_CACHE:
        _NC_CACHE[nb] = build_nc(nb)
    return _NC_CACHE[nb]


def make_in_maps(x, We, be, Wq, bq, Wk, bk, Wv, bv, W2, b2, W3, b3,
                 ncores=NCORES):
    x = np.ascontiguousarray(x, np.float32)
    if x.max() > 1.0:
        x = (x * np.float32(1.0 / 255.0)).astype(np.float32)

    def _pad128(w):  # pad [DEMB, DQK] -> [DEMB, 128] so LDWEIGHTS can FWL
        p = np.zeros((w.shape[0], 128), np.float32)
        p[:, :w.shape[1]] = w
        return p

    def _pack_blocks(w, rchs, cchs):
        """flatten [R, C] into contiguous (r-chunk, c-chunk) blocks"""
        return np.concatenate(
            [w[r0:r0 + rn, c0:c0 + cn].ravel()
             for (r0, rn) in rchs for (c0, cn) in cchs])

    weh, wel = _split(np.ascontiguousarray(We.T))
    wS = _pack_blocks(np.concatenate([weh, weh, wel], 0), CH_KS, CH_EMB)
    wQKp = round_m11(np.concatenate([Wq.T, Wk.T], 1))  # [DEMB, 128]
    wvh, wvl = _split(np.ascontiguousarray(Wv.T))
    w2h, w2l = _split(np.ascontiguousarray(W2.T))
    w2h = _pack_blocks(w2h, CH_EMB, CH_H2)
    w2l = _pack_blocks(w2l, CH_EMB, CH_H2)
    w3h, w3l = _split(np.ascontiguousarray(W3.T))

    # bV compensation: fold the batch-mean of the dropped s1 @ Wv-lo term
    # into the bias (spike rates from a host embed forward).
    em = (x.reshape(-1, DIN) @ We.T.astype(np.float32)) + be
    pbar = (em > 0.5).mean(0, dtype=np.float64).astype(np.float32)
    del em
    bv_c = (bv.astype(np.float32) + pbar @ wvl).astype(np.float32)

    shared = dict(
        wS=wS, wQK=np.ascontiguousarray(wQKp), wVh=wvh, w2h=w2h, w2l=w2l,
        w3h=w3h, w3l=w3l,
        bE=np.ascontiguousarray(be.reshape(-1, 1), np.float32),
        ones=np.ones((128, 1), np.float32),
        z64=np.zeros((64, 128), np.float32),
        bQK=np.ascontiguousarray(
            np.concatenate([bq, bk]).reshape(-1, 1), np.float32),
        bV=np.ascontiguousarray(bv_c.reshape(-1, 1), np.float32),
        b2=np.ascontiguousarray(b2.reshape(-1, 1), np.float32),
        b3=np.ascontiguousarray(b3.reshape(-1, 1), np.float32),
    )
    nb = x.shape[0] // ncores
    in_maps = []
    for c in range(ncores):
        xs = x[c * nb:(c + 1) * nb]                       # [nb, S, DIN]
        xT = np.ascontiguousarray(xs.transpose(0, 2, 1))  # [nb, DIN, S]
        xh_, xl_ = _split(xT)
        xpk_ = np.concatenate([xh_, xl_, xh_], axis=1)    # [nb, 2352, S]
        in_maps.append(dict(shared, xpk=np.ascontiguousarray(xpk_)))
    return in_maps, nb


def kernel(x, We, be, Wq, bq, Wk, bk, Wv, bv, W2, b2, W3, b3, _trace=False):
    args = [np.asarray(a, np.float32) for a in
            (x, We, be, Wq, bq, Wk, bk, Wv, bv, W2, b2, W3, b3)]
    in_maps, nb = make_in_maps(*args)
    nc = _get_nc(nb)
    res = run_bass_kernel_spmd(nc, in_maps, list(range(NCORES)), trace=_trace)
    spk3 = np.concatenate([r["os"].transpose(0, 2, 1) for r in res.results], 0)
    mem3 = np.concatenate([r["om"].transpose(0, 2, 1) for r in res.results], 0)
    kernel.last_results = res
    return (np.ascontiguousarray(spk3, np.float32),
            np.ascontiguousarray(mem3, np.float32))


# revision 46
# speedup vs baseline: 1.1309x; 1.1309x over previous
"""Trainium2 Bass kernel for nn_AttentionSpikingNetwork (B=64, S=512).

Data-parallel over batch across 8 NeuronCores (8 batch elems per core).
All matmuls run as float32r (FP22, full PE rate). PE-work-minimized pass
structure (validated against the reference inputs in a numpy FP22 emulator;
rel err 2.6e-3, zero spike flips, stable under accumulation-order noise):
  - embed (threshold-critical): exact 3-term hi/lo (Weh*xh + Weh*xl +
    Wel*xh), emitted as one packed K=2352 stream [xh; xl; xh] against
    [Weh; Weh; Wel] so partial chunks merge: 95 matmuls/elem instead of
    the naive 105.
  - Q/K: one packed single-pass matmul group (Q cols 0:64, K cols 64:128);
    the K rows are DMA-shifted to partitions 0:64 of persistent zero-padded
    tiles so scores stay in the fast K=128 tile mode.
  - scores: single pass (softmax normalization cancels FP22 rounding).
  - V: single pass (Wv-hi only); the dropped s1*Wv-lo term's batch mean is
    folded into bv on the host (spike rates from a cheap host embed
    forward), measured 2.4x error reduction.
  - attention: single pass on the FP22-rounded V (the rounding noise is
    averaged away by the attention weights; measured exact-class).
  - cur2 (threshold-critical): exact 3-pass hi/lo.
  - cur3: exact 2-pass (spikes are FP22-exact).
Activations flow transposed ([feat, seq]) so biases/thresholds fuse into
single per-partition DVE ops reading PSUM. Scores are produced transposed
(K @ Q.T); the softmax runs without max-subtraction and its denominator
comes from PE ones-matmuls off the critical path. Batch element b+1's
embed chunk-groups are interleaved through b's entire attention phase so
x-chunk consumption matches the uniform DMA delivery rate and every
latency chain (exp, reciprocal, normalize, spike drains) hides under
embed matmuls; the last element's qk/V/scores/den are hoisted into the
previous phase so only its attention tail remains unoverlapped. DMA
trigger queues are chosen so nothing latency-critical (exp on scalar,
invb broadcast on gpsimd) sits behind bulk transfer triggers.
"""
import os
import sys

for _p in ("/opt/trn_rl_repo", "/root/.axon_site/_ro/trn_rl_repo"):
    if os.path.isdir(_p) and _p not in sys.path:
        sys.path.insert(0, _p)

import numpy as np
from contextlib import ExitStack

import concourse.bass as bass
import concourse.bass_isa as bass_isa
import concourse.bacc as bacc
import concourse.mybir as mybir
import concourse.tile as tile
from concourse.bass_utils import run_bass_kernel_spmd

F32 = mybir.dt.float32
F32R = mybir.dt.float32r
AF = mybir.ActivationFunctionType
OP = mybir.AluOpType

NCORES = 8
B, S, DIN, DEMB, DQK, DH2, DOUT = 64, 512, 784, 600, 64, 200, 10
NB = B // NCORES  # batch elems per core

def _chunks(total, step=128):
    return [(i, min(step, total - i)) for i in range(0, total, step)]

KSTACK = 3 * DIN          # packed [xh; xl; xh] contraction length
CH_KS = _chunks(KSTACK)   # 19 chunks (18x128 + 48)
CH_EMB = _chunks(DEMB)    # 5
CH_H2 = _chunks(DH2)      # 2
CH_S = _chunks(S)         # 4
CH_VN = [(0, 344), (344, 256)]  # V free-dim split; both >=256 keeps fp32r full-rate


def round_m11(a):
    """Round fp32 to 11 explicit mantissa bits (fp32r/FP22 grid), RNE."""
    a = np.ascontiguousarray(a, np.float32)
    u = a.view(np.uint32).astype(np.uint64)
    r = (u + 0x7FF + ((u >> 12) & 1)) & np.uint64(0xFFFFF000)
    return r.astype(np.uint32).view(np.float32)


def _split(a):
    hi = round_m11(a)
    lo = (a.astype(np.float32) - hi).astype(np.float32)
    return hi, lo


def build_nc(nb=NB):
    nc = bacc.Bacc()

    def par(name, shape, dt=F32R, out=False):
        return nc.declare_dram_parameter(name, list(shape), dt, isOutput=out)

    xpk = par("xpk", [nb, KSTACK, S])
    wS = par("wS", [KSTACK * DEMB])
    wQK = par("wQK", [DEMB, 128])
    wVh = par("wVh", [DEMB, DEMB])
    w2h = par("w2h", [DEMB * DH2]); w2l = par("w2l", [DEMB * DH2])
    w3h = par("w3h", [DH2, DOUT]); w3l = par("w3l", [DH2, DOUT])
    bE = par("bE", [DEMB, 1], F32); bQK = par("bQK", [128, 1], F32)
    bV = par("bV", [DEMB, 1], F32)
    b2 = par("b2", [DH2, 1], F32); b3 = par("b3", [DOUT, 1], F32)
    ones = par("ones", [128, 1])
    z64 = par("z64", [64, 128])
    os_ = par("os", [nb, DOUT, S], F32, out=True)
    om_ = par("om", [nb, DOUT, S], F32, out=True)

    with ExitStack() as ctx:
        tc = ctx.enter_context(tile.TileContext(nc))
        wp = ctx.enter_context(tc.tile_pool(name="wp", bufs=1))
        xp = ctx.enter_context(tc.tile_pool(name="xp", bufs=12))
        sp = ctx.enter_context(tc.tile_pool(name="sp", bufs=1))
        outp = ctx.enter_context(tc.tile_pool(name="outp", bufs=1))
        ps_em = ctx.enter_context(tc.tile_pool(name="ps_em", bufs=1, space="PSUM"))
        ps = ctx.enter_context(tc.tile_pool(name="ps", bufs=3, space="PSUM"))

        # ---- resident weights / consts ----
        # DMA emission order is load order: the packed embed weight blocks
        # stream in per-k-chunk interleaved with b=0's x chunks so the first
        # matmul starts after ~0.5MB. Everything else loads during b=0's
        # embed compute (see _load_rest below).
        def blocks2(dram, rchs, cchs, nm, dma=True):
            """dedicated [rn, cn] weight blocks, host-packed contiguously"""
            out = {}
            off = 0
            for i, (r0, rn) in enumerate(rchs):
                for j, (c0, cn) in enumerate(cchs):
                    t = wp.tile([rn, cn], F32R, name=f"{nm}_{i}_{j}",
                                tag=f"{nm}_{i}_{j}")
                    out[(i, j)] = (t, off, rn, cn)
                    if dma:
                        nc.scalar.dma_start(
                            out=t, in_=dram[off:off + rn * cn].rearrange(
                                "(a b) -> a b", b=cn))
                    off += rn * cn
            return out

        def wtiles(dram, chs, width, nm, dma=True):
            hs = []
            for i, (c0, cn) in enumerate(chs):
                t = wp.tile([cn, width], F32R, name=f"{nm}{i}", tag=f"{nm}{i}")
                if dma:
                    nc.scalar.dma_start(out=t, in_=dram[c0:c0 + cn, :])
                hs.append(t)
            return hs

        def btiles(dram, chs, nm):
            hs = []
            for i, (c0, cn) in enumerate(chs):
                t = wp.tile([cn, 1], F32, name=f"{nm}{i}", tag=f"{nm}{i}")
                nc.scalar.dma_start(out=t, in_=dram[c0:c0 + cn, :])
                hs.append(t)
            return hs

        wS_m = blocks2(wS, CH_KS, CH_EMB, "wS", dma=False)
        _rest = {}

        def _load_rest():
            _rest["wQK"] = wtiles(wQK, CH_EMB, 128, "wQK")
            _rest["bQK"] = btiles(bQK, [(0, 128)], "bQK")[0]
            _rest["wVh"] = wtiles(wVh, CH_EMB, DEMB, "wVh")
            _rest["bV"] = btiles(bV, CH_EMB, "bV")
            _rest["w2h"] = {k: v[0] for k, v in
                            blocks2(w2h, CH_EMB, CH_H2, "w2h").items()}
            _rest["w2l"] = {k: v[0] for k, v in
                            blocks2(w2l, CH_EMB, CH_H2, "w2l").items()}
            _rest["b2"] = btiles(b2, CH_H2, "b2")
            _rest["w3h"] = wtiles(w3h, CH_H2, DOUT, "w3h")
            _rest["w3l"] = wtiles(w3l, CH_H2, DOUT, "w3l")
            _rest["b3"] = btiles(b3, [(0, DOUT)], "b3")[0]

        bE_t = btiles(bE, CH_EMB, "bE")
        ones_t = wp.tile([128, 1], F32R, name="ones_t", tag="ones_t")
        nc.scalar.dma_start(out=ones_t, in_=ones[:, :])

        # Persistent scores-lhsT tiles: rows 0:64 get each elem's K slice
        # (DMA'd down from the packed QK drain), rows 64:128 stay zero so
        # the scores matmul runs at the fast K=128 tile mode.
        kh_t = []
        for j, (t0, tn) in enumerate(CH_S):
            kh = wp.tile([128, tn], F32R, name=f"kh{j}", tag=f"kh{j}")
            nc.scalar.dma_start(out=kh[64:128, :], in_=z64[:, 0:tn])
            kh_t.append(kh)

        MM = nc.tensor.matmul

        # Software pipeline: elem b+1's embed matmuls are emitted between
        # elem b's scores and its softmax-sum/attention matmuls, giving the
        # PE ~20us of independent work while ACT/DVE run b's exp chain.
        st = [dict() for _ in range(nb)]

        def emit_embed_start(b):
            em_ps = []
            for i, (c0, cn) in enumerate(CH_EMB):
                t = ps_em.tile([cn, S], F32, name=f"em{i}", tag=f"em{i}")
                em_ps.append(t)
            st[b]["em_ps"] = em_ps
            st[b]["xt"] = {}

        def prefetch_x(b, q, kidx):
            # issue x-chunk DMAs well ahead of their matmuls. Allocation in
            # consumption order keeps the xp buffer cycle aligned with the
            # matmul order (chunk k+11 reuses chunk k's buffer, which is
            # long consumed). The gpsimd-queue portion is issued only after
            # den: a DMA trigger occupies its queue for ~0.7us, and the
            # latency-critical invb broadcast must not sit behind them.
            xt = st[b]["xt"]
            for k in kidx:
                k0, kn = CH_KS[k]
                t = xp.tile([kn, S], F32R, name=f"x{k}", tag="x_t")
                q.dma_start(out=t, in_=xpk[b, k0:k0 + kn, :])
                xt[k] = t

        def emit_embed_stack(b, kidx):
            em_ps = st[b]["em_ps"]
            last = len(CH_KS) - 1
            for k in kidx:
                if b == 0:
                    # b=0 is HBM-bound (~10.4MB before the embed ends), so
                    # emission order is per-chunk across all three queues:
                    # weight blocks alternate scalar/gpsimd, x chunks
                    # alternate sync/gpsimd, so every queue's delivery of
                    # chunk k's data slightly precedes the PE's need for it
                    for j in range(len(CH_EMB)):
                        t, off, rn, cn_ = wS_m[(k, j)]
                        q = (nc.scalar, nc.gpsimd, nc.sync)[(k + j) % 3]
                        q.dma_start(
                            out=t, in_=wS[off:off + rn * cn_].rearrange(
                                "(a b) -> a b", b=cn_))
                    k0, kn = CH_KS[k]
                    t = xp.tile([kn, S], F32R, name=f"x{k}", tag="x_t")
                    (nc.sync if k % 2 == 0 else nc.gpsimd).dma_start(
                        out=t, in_=xpk[b, k0:k0 + kn, :])
                    st[b]["xt"][k] = t
                x_t = st[b]["xt"][k]
                for j in range(len(CH_EMB)):
                    MM(em_ps[j], wS_m[(k, j)][0], x_t,
                       start=(k == 0), stop=(k == last))
            if b == 0 and 0 in kidx:
                _load_rest()

        def emit_embed_drain(b):
            em_ps = st[b]["em_ps"]
            s1_t = []
            for i, (c0, cn) in enumerate(CH_EMB):
                t = sp.tile([cn, S], F32R, name=f"s1_{i}", tag=f"s1_{i}", bufs=2)
                nc.vector.tensor_scalar(t, em_ps[i], bE_t[i], 0.5, OP.add, OP.is_gt)
                s1_t.append(t)
            st[b]["s1"] = s1_t

        def emit_qk(b):
            s1_t = st[b]["s1"]
            wQK_t = _rest["wQK"]

            # Packed Q|K single pass (Q cols 0:64, K cols 64:128): one
            # 5-matmul group instead of two. Scores single-pass FP22 (the
            # softmax normalization cancels the common-mode rounding).
            qk_ps = ps.tile([128, S], F32, name="qk_ps", tag="ps")
            n = len(CH_EMB)
            for i in range(n):
                MM(qk_ps, wQK_t[i], s1_t[i], start=(i == 0),
                   stop=(i == n - 1))
            qh_t = sp.tile([128, S], F32R, name="qh", tag="qh")
            nc.vector.tensor_scalar(qh_t, qk_ps, _rest["bQK"], None, OP.add)
            # K rows shift down to partitions 0:64 of the persistent kh
            # tiles (rows 64:128 zero); qh_t itself is the scores rhs — its
            # K rows 64:128 meet the kh zeros. Triggered from the sync
            # queue, ahead of the x prefetch: the trigger blocks its queue
            # until the drain lands, and both the scalar queue (exp) and
            # gpsimd queue (invb broadcast) have latency-critical work.
            for j, (t0, tn) in enumerate(CH_S):
                nc.sync.dma_start(out=kh_t[j][0:64, :],
                                  in_=qh_t[64:128, t0:t0 + tn])

            st[b].update(kh=kh_t, qh=qh_t)

        def emit_V(b, chs=None, append=False):
            s1_t = st[b]["s1"]
            wVh_t = _rest["wVh"]
            # V natural = spk1 @ Wvh.T (single pass; the dropped Wv-lo term's
            # mean is compensated in bV host-side). QK psum drains hide here.
            vh_t = st[b]["vh"] if append else []
            base = len(vh_t)
            for dti, (t0, tn) in enumerate(chs if chs is not None else CH_S):
                ti = base + dti
                v_ps = [ps.tile([tn, w], F32, name=f"v_ps{j}", tag="ps")
                        for j, (v0, w) in enumerate(CH_VN)]
                n = len(CH_EMB)
                for i in range(n):
                    lh = s1_t[i][:, t0:t0 + tn]
                    for j, (v0, w) in enumerate(CH_VN):
                        MM(v_ps[j], lh, wVh_t[i][:, v0:v0 + w],
                           start=(i == 0), stop=(i == n - 1))
                vh = sp.tile([tn, DEMB], F32R, name=f"vh{ti}", tag=f"vh{ti}")
                for j, (v0, w) in enumerate(CH_VN):
                    nc.scalar.activation(vh[:, v0:v0 + w], v_ps[j], AF.Copy)
                vh_t.append(vh)

            st[b]["vh"] = vh_t

        def emit_scores(b):
            qh_t, kh_t = st[b]["qh"], st[b]["kh"]
            # scores.T = K @ Q.T (single-pass FP22) + exp, per t-chunk.
            # exp writes the f32r attention operand directly from PSUM.
            pth_t = []
            for ti, (t0, tn) in enumerate(CH_S):
                scT_ps = ps.tile([tn, S], F32, name=f"scT_ps{ti}", tag="ps")
                MM(scT_ps, kh_t[ti], qh_t, start=True, stop=True)
                ph = sp.tile([tn, S], F32R, name=f"pth{ti}", tag=f"pth{ti}")
                nc.scalar.activation(ph, scT_ps, AF.Exp, scale=0.125)
                pth_t.append(ph)
            st[b].update(pth=pth_t)

        def emit_den(b):
            # softmax denominator = Sum_t P.T via PE ones-matmuls (the same
            # rounded operand the attention numerator uses). Emitted
            # mid-embed-filler so the slow DVE reciprocal lands well before
            # the spk2_in stage needs invb.
            pth_t = st[b]["pth"]
            den_ps = ps.tile([1, S], F32, name="den_ps", tag="ps")
            nt = len(CH_S)
            for ti in range(nt):
                MM(den_ps, ones_t[0:CH_S[ti][1], :], pth_t[ti],
                   start=(ti == 0), stop=(ti == nt - 1))
            invs = sp.tile([1, S], F32, name="invs", tag="invs", bufs=2)
            nc.vector.reciprocal(invs, den_ps)
            invb = sp.tile([128, S], F32, name="invb", tag="invb")
            nc.gpsimd.partition_broadcast(invb, invs)
            st[b]["invb"] = invb

        # attn/cur2 chunk order: the 88-row chunk first, so its DVE chain
        # (the last cur2 passes depend on it) starts earliest
        IORDER = [len(CH_EMB) - 1] + list(range(len(CH_EMB) - 1))

        def emit_attn_chunk(b, i):
            # attn_out.T = V.T @ P.T (single pass); + normalize + bv + spk1.T
            vh_t, pth_t = st[b]["vh"], st[b]["pth"]
            invb = st[b]["invb"]
            nt = len(CH_S)
            c0, cn = CH_EMB[i]
            ao_ps = ps.tile([cn, S], F32, name=f"ao_ps{i}", tag="ps")
            for ti in range(nt):
                MM(ao_ps, vh_t[ti][:, c0:c0 + cn], pth_t[ti],
                   start=(ti == 0), stop=(ti == nt - 1))
            raw = sp.tile([cn, S], F32, name="s2raw", tag="s2raw", bufs=2)
            nc.vector.scalar_tensor_tensor(raw, ao_ps, 0.0, invb[0:cn, :],
                                           OP.add, OP.mult)
            nc.vector.scalar_tensor_tensor(raw, raw, _rest["bV"][i],
                                           st[b]["s1"][i].bitcast(F32),
                                           OP.add, OP.add)
            h = sp.tile([cn, S], F32R, name=f"s2h{i}", tag=f"s2h{i}")
            l = sp.tile([cn, S], F32R, name=f"s2l{i}", tag=f"s2l{i}")
            nc.scalar.activation(h, raw, AF.Copy)
            nc.vector.tensor_tensor(l, raw, h.bitcast(F32), OP.subtract)
            st[b].setdefault("s2h", {})[i] = h
            st[b].setdefault("s2l", {})[i] = l

        def emit_cur2_part(b, hi):
            # cur2.T = W2 @ spk2_in.T (exact 3 passes) -> spk2
            s2h_t, s2l_t = st[b]["s2h"], st[b]["s2l"]
            h0, hn = CH_H2[hi]
            c2_ps = ps.tile([hn, S], F32, name=f"c2_ps{hi}", tag="ps")
            for oi, i in enumerate(IORDER):
                wh = _rest["w2h"][(i, hi)]
                wl = _rest["w2l"][(i, hi)]
                MM(c2_ps, wh, s2h_t[i], start=(oi == 0), stop=False)
                MM(c2_ps, wh, s2l_t[i], start=False, stop=False)
                MM(c2_ps, wl, s2h_t[i],
                   start=False, stop=(oi == len(IORDER) - 1))
            t = sp.tile([hn, S], F32R, name=f"spk2_{hi}", tag=f"spk2_{hi}")
            nc.vector.tensor_scalar(t, c2_ps, _rest["b2"][hi], 0.3,
                                    OP.add, OP.is_gt)
            st[b].setdefault("s2", {})[hi] = t

        def emit_cur3_out(b):
            # cur3.T = W3 @ spk2.T (exact 2 passes) -> outputs
            s2_t = st[b]["s2"]
            c3_ps = ps.tile([DOUT, S], F32, name="c3_ps", tag="ps")
            n = len(CH_H2)
            for hi in range(n):
                MM(c3_ps, _rest["w3h"][hi], s2_t[hi], start=(hi == 0), stop=False)
                MM(c3_ps, _rest["w3l"][hi], s2_t[hi], start=False, stop=(hi == n - 1))
            spk3_t = outp.tile([DOUT, S], F32, name="spk3_t", tag="spk3_t")
            c3b_t = outp.tile([DOUT, S], F32, name="c3b_t", tag="c3b_t")
            mem3_t = outp.tile([DOUT, S], F32, name="mem3_t", tag="mem3_t")
            nc.vector.tensor_scalar(spk3_t, c3_ps, _rest["b3"], 0.3, OP.add, OP.is_gt)
            nc.vector.tensor_scalar(c3b_t, c3_ps, _rest["b3"], None, OP.add)
            nc.vector.scalar_tensor_tensor(mem3_t, spk3_t, -0.3, c3b_t,
                                           OP.mult, OP.add)
            nc.scalar.dma_start(out=os_[b, :, :], in_=spk3_t)
            nc.scalar.dma_start(out=om_[b, :, :], in_=mem3_t)

        NKS = len(CH_KS)
        emit_embed_start(0)
        emit_embed_stack(0, range(NKS))
        emit_embed_drain(0)
        for b in range(nb):
            if b == nb - 1:
                # qk/V/scores/den were hoisted into b-1's phase; only the
                # attention tail remains here.
                for i in IORDER:
                    emit_attn_chunk(b, i)
                emit_cur2_part(b, 0)
                emit_cur2_part(b, 1)
                emit_cur3_out(b)
                break
            emit_qk(b)
            # issue b+1's first 11 x DMAs now: the PE consumes them
            # ~35us from here, so the sync queue never starves the
            # embed matmuls. The rest go on gpsimd after den.
            emit_embed_start(b + 1)
            prefetch_x(b + 1, nc.sync, range(11))
            emit_V(b)
            # b+1's embed chunk-groups are interleaved through b's whole
            # attention phase: x-chunk consumption is then spread evenly,
            # matching the uniform DMA delivery rate, and every latency
            # chain (exp, reciprocal, normalize, spike drains) hides under
            # embed matmuls.
            emit_embed_stack(b + 1, [0])
            emit_scores(b)
            emit_embed_stack(b + 1, [1])
            emit_den(b)
            prefetch_x(b + 1, nc.gpsimd, range(11, NKS))
            emit_embed_stack(b + 1, [2, 3])
            for n_at, i in enumerate(IORDER):
                emit_attn_chunk(b, i)
                emit_embed_stack(b + 1, range(4 + 2 * n_at, 6 + 2 * n_at))
            emit_cur2_part(b, 0)
            emit_embed_stack(b + 1, range(14, 17))
            emit_cur2_part(b, 1)
            emit_embed_stack(b + 1, range(17, NKS))
            emit_cur3_out(b)
            emit_embed_drain(b + 1)
            if b + 1 == nb - 1:
                # hoist the last element's pre-attention stages here so its
                # exp/reciprocal chains hide under this phase's matmuls
                emit_qk(b + 1)
                emit_V(b + 1)
                emit_scores(b + 1)
                emit_den(b + 1)

    nc.finalize()
    return nc


_NC_CACHE = {}


def _get_nc(nb):
    if nb not in _NC_CACHE:
        _NC_CACHE[nb] = build_nc(nb)
    return _NC_CACHE[nb]


def make_in_maps(x, We, be, Wq, bq, Wk, bk, Wv, bv, W2, b2, W3, b3,
                 ncores=NCORES):
    x = np.ascontiguousarray(x, np.float32)
    if x.max() > 1.0:
        x = (x * np.float32(1.0 / 255.0)).astype(np.float32)

    def _pad128(w):  # pad [DEMB, DQK] -> [DEMB, 128] so LDWEIGHTS can FWL
        p = np.zeros((w.shape[0], 128), np.float32)
        p[:, :w.shape[1]] = w
        return p

    def _pack_blocks(w, rchs, cchs):
        """flatten [R, C] into contiguous (r-chunk, c-chunk) blocks"""
        return np.concatenate(
            [w[r0:r0 + rn, c0:c0 + cn].ravel()
             for (r0, rn) in rchs for (c0, cn) in cchs])

    weh, wel = _split(np.ascontiguousarray(We.T))
    wS = _pack_blocks(np.concatenate([weh, weh, wel], 0), CH_KS, CH_EMB)
    wQKp = round_m11(np.concatenate([Wq.T, Wk.T], 1))  # [DEMB, 128]
    wvh, wvl = _split(np.ascontiguousarray(Wv.T))
    w2h, w2l = _split(np.ascontiguousarray(W2.T))
    w2h = _pack_blocks(w2h, CH_EMB, CH_H2)
    w2l = _pack_blocks(w2l, CH_EMB, CH_H2)
    w3h, w3l = _split(np.ascontiguousarray(W3.T))

    # bV compensation: fold the batch-mean of the dropped s1 @ Wv-lo term
    # into the bias (spike rates from a host embed forward).
    em = (x.reshape(-1, DIN) @ We.T.astype(np.float32)) + be
    pbar = (em > 0.5).mean(0, dtype=np.float64).astype(np.float32)
    del em
    bv_c = (bv.astype(np.float32) + pbar @ wvl).astype(np.float32)

    shared = dict(
        wS=wS, wQK=np.ascontiguousarray(wQKp), wVh=wvh, w2h=w2h, w2l=w2l,
        w3h=w3h, w3l=w3l,
        bE=np.ascontiguousarray(be.reshape(-1, 1), np.float32),
        ones=np.ones((128, 1), np.float32),
        z64=np.zeros((64, 128), np.float32),
        bQK=np.ascontiguousarray(
            np.concatenate([bq, bk]).reshape(-1, 1), np.float32),
        bV=np.ascontiguousarray(bv_c.reshape(-1, 1), np.float32),
        b2=np.ascontiguousarray(b2.reshape(-1, 1), np.float32),
        b3=np.ascontiguousarray(b3.reshape(-1, 1), np.float32),
    )
    nb = x.shape[0] // ncores
    in_maps = []
    for c in range(ncores):
        xs = x[c * nb:(c + 1) * nb]                       # [nb, S, DIN]
        xT = np.ascontiguousarray(xs.transpose(0, 2, 1))  # [nb, DIN, S]
        xh_, xl_ = _split(xT)
        xpk_ = np.concatenate([xh_, xl_, xh_], axis=1)    # [nb, 2352, S]
        in_maps.append(dict(shared, xpk=np.ascontiguousarray(xpk_)))
    return in_maps, nb


def kernel(x, We, be, Wq, bq, Wk, bk, Wv, bv, W2, b2, W3, b3, _trace=False):
    args = [np.asarray(a, np.float32) for a in
            (x, We, be, Wq, bq, Wk, bk, Wv, bv, W2, b2, W3, b3)]
    in_maps, nb = make_in_maps(*args)
    nc = _get_nc(nb)
    res = run_bass_kernel_spmd(nc, in_maps, list(range(NCORES)), trace=_trace)
    spk3 = np.concatenate([r["os"].transpose(0, 2, 1) for r in res.results], 0)
    mem3 = np.concatenate([r["om"].transpose(0, 2, 1) for r in res.results], 0)
    kernel.last_results = res
    return (np.ascontiguousarray(spk3, np.float32),
            np.ascontiguousarray(mem3, np.float32))
